# revision 2
# baseline (speedup 1.0000x reference)
"""Trainium2 Bass kernel for nn_Attention_87892210745803 (v2).

Full problem: x [4, 2048, 1024] fp32 -> fused QKV projection (W_qkv [3072, 1024],
b_qkv [3072]) -> 16-head causal attention (head size 64) -> out [4, 2048, 1024].

Sharding (8 cores): core c handles batch b = c // 2 and head-group g = c % 2
(8 of the 16 heads). The host pre-transposes and pre-casts the per-core
inputs: xT = x[b].T (bf16 [E, T]) and wT = W_c.T (bf16 [E, 3F]) so the device
needs no PE transposes at all; biases stay fp32.

Per-core kernel (Bass/Tile, bf16 matmuls, fp32 psum):
  phase 1 (QKV): per t-block of 512: DMA xT tiles; q^T/k^T with f on
           partitions (attention-ready, bias added in the psum->SBUF DVE
           copy); v in natural [t, f] layout with a ones-column appended
           (softmax denominator falls out of the o-matmul); v bias is folded
           into the final output add (softmax weights sum to 1).
  phase 2 (attention) per i-block of 512: per head pair, s^T = k^T q on PE
           (the pair's two K=64 matmuls sit at partition bases 0/64 ->
           disjoint PE row groups), exp on ACT (diagonal windows shrunk to
           the valid >=128-wide i-range), causal zeroing of the single
           diagonal 128x128 block via gpsimd affine_select, then o[i, d]
           accumulated directly with ex as the stationary operand
           ([128 j, 128 i] x [128 j, 65]) -> out free dim is only 65 wide,
           i lands on partitions, so no transpose epilogue: one reciprocal +
           broadcast-mult per head and a bias add per i-tile.

The two phases are software-pipelined (attention i-block I is emitted right
after QKV t-block I) to keep PE dense through the ACT-bound exp stretches.

Timing note: per-iteration HW time is measured in test.py by building this
kernel with an in-kernel For_i repeat loop (reps=5 vs 25) because per-dispatch
axon overhead (tens of ms) swamps the kernel time.
"""

import sys

sys.path.insert(0, "/opt/trn_rl_repo")

import numpy as np

B, T, E = 4, 2048, 1024
NH_GLOBAL = 16
HS = 64
P = 128
N_CORES = 8
H = 8  # heads per core
F = H * HS  # 512: rows per q/k/v block per core

_CACHE = {}


def _build_nc(
    T=T,
    E=E,
    H=H,
    IB=512,
    reps=1,
    big_bufs=2,
    sp_bufs=2,
    ops_bufs=4,
    xt_bufs=2,
    ex_bufs=6,
    outsb_bufs=2,
    interleave=True,
):
    import contextlib

    import concourse.bacc as bacc
    import concourse.mybir as mybir
    import concourse.tile as tile

    F32 = mybir.dt.float32
    BF16 = mybir.dt.bfloat16
    F = H * HS
    EO = E // P  # contraction subtiles for QKV
    TT = T // P  # t-tiles
    FQK = 2 * F // P  # f-tiles for q+k
    TBS = min(IB, 512)  # t-block size for phase 1
    NTB = T // TBS
    NI = T // IB
    JPI = IB // P
    assert not interleave or IB == TBS

    nc = bacc.Bacc("TRN2", target_bir_lowering=False, debug=False)
    xT_d = nc.dram_tensor("xT", [E, T], BF16, kind="ExternalInput").ap()
    w_d = nc.dram_tensor("w", [E, 3 * F], BF16, kind="ExternalInput").ap()
    b_d = nc.dram_tensor("b", [3 * F], F32, kind="ExternalInput").ap()
    out_d = nc.dram_tensor("out", [T, F], F32, kind="ExternalOutput").ap()

    with tile.TileContext(nc) as tc:
        with (
            tc.tile_pool(name="const", bufs=1) as const_pool,
            tc.tile_pool(name="persist", bufs=1) as persist,
            tc.tile_pool(name="wT", bufs=1) as wT_pool,
            tc.tile_pool(name="xT", bufs=xt_bufs) as xT_pool,
            tc.tile_pool(name="exp", bufs=ex_bufs) as exp_pool,
            tc.tile_pool(name="recip", bufs=4) as recip_pool,
            tc.tile_pool(name="outsb", bufs=outsb_bufs) as out_pool,
            tc.tile_pool(name="big", bufs=big_bufs, space="PSUM") as big_pool,
            tc.tile_pool(name="sp", bufs=sp_bufs, space="PSUM") as sp_pool,
            tc.tile_pool(name="ops", bufs=ops_bufs, space="PSUM") as ops_pool,
        ):
            b_sb = const_pool.tile([P, FQK], F32)
            nc.sync.dma_start(b_sb[:], b_d[0 : 2 * F].rearrange("(o p) -> p o", p=P))
            bias_v = const_pool.tile([P, F], F32)
            nc.sync.dma_start(
                bias_v[:], b_d[None, 2 * F : 3 * F].to_broadcast((P, F))
            )

            qkT = persist.tile([P, FQK, T], BF16)
            v_aug = persist.tile([P, TT, H, HS + 1], BF16)
            ones_col = const_pool.tile([P, 1], F32)
            nc.vector.memset(ones_col, 1.0)
            nc.vector.tensor_copy(
                v_aug[:, :, :, HS : HS + 1],
                ones_col[:, None, None, :].to_broadcast((P, TT, H, 1)),
            )

            rep_ctx = tc.For_i(0, reps, 1) if reps > 1 else contextlib.nullcontext()
            with rep_ctx:
                wT = wT_pool.tile([P, EO, 3 * F], BF16)
                for eo in range(EO):
                    nc.sync.dma_start(wT[:, eo, :], w_d[eo * P : (eo + 1) * P, :])

                # ============ phase 1: QKV projection ============
                def p1_tblock(tb):
                    xT = xT_pool.tile([P, EO, TBS], BF16, tag="xT", name="xT")
                    for eo in range(EO):
                        nc.sync.dma_start(
                            xT[:, eo, :],
                            xT_d[eo * P : (eo + 1) * P, tb * TBS : (tb + 1) * TBS],
                        )
                    # q^T / k^T tiles: psum[f=128, t=TBS], bias in the copy-out
                    for wf in range(FQK):
                        ps = big_pool.tile([P, 512], F32, tag="big", name="qkps")[
                            :, :TBS
                        ]
                        for eo in range(EO):
                            nc.tensor.matmul(
                                ps,
                                wT[:, eo, wf * P : (wf + 1) * P],
                                xT[:, eo, :],
                                start=(eo == 0),
                                stop=(eo == EO - 1),
                            )
                        nc.vector.tensor_scalar_add(
                            qkT[:, wf, tb * TBS : (tb + 1) * TBS],
                            ps,
                            b_sb[:, wf : wf + 1],
                        )
                    # v tiles: psum[t=128, f=F]; bias folded into the final add
                    for tt in range(TBS // P):
                        git = tb * (TBS // P) + tt
                        ps = big_pool.tile([P, 512], F32, tag="big", name="vps")[
                            :, :F
                        ]
                        for eo in range(EO):
                            nc.tensor.matmul(
                                ps,
                                xT[:, eo, tt * P : (tt + 1) * P],
                                wT[:, eo, 2 * F : 3 * F],
                                start=(eo == 0),
                                stop=(eo == EO - 1),
                            )
                        nc.vector.tensor_copy(
                            v_aug[:, git, :, 0:HS],
                            ps.rearrange("p (h d) -> p h d", d=HS),
                        )

                # ============ phase 2: attention ============
                def p2_iblock(I):
                    out_sb = out_pool.tile(
                        [P, JPI, F], F32, tag="outsb", name="out_sb"
                    )
                    njt = JPI * (I + 1)
                    for hp in range(H // 2):
                        fq = hp
                        fk = H // 2 + hp
                        ops_pair = [
                            ops_pool.tile(
                                [P, JPI, HS + 1], F32, tag="ops", name="ops_t"
                            )
                            for _ in range(2)
                        ]
                        for jt in range(njt):
                            r = jt - JPI * I  # >= 0 on the diagonal j-tiles
                            off = max(0, P * r)
                            w = IB - off
                            exs = []
                            for half in range(2):
                                pb = half * HS
                                sp = sp_pool.tile(
                                    [P, 512], F32, tag="sp", name="sp"
                                )[:, :w]
                                nc.tensor.matmul(
                                    sp,
                                    qkT[pb : pb + HS, fk, jt * P : (jt + 1) * P],
                                    qkT[
                                        pb : pb + HS,
                                        fq,
                                        I * IB + off : (I + 1) * IB,
                                    ],
                                    start=True,
                                    stop=True,
                                )
                                ex = exp_pool.tile(
                                    [P, IB], BF16, tag="exp", name="ex"
                                )[:, :w]
                                nc.scalar.activation(
                                    ex,
                                    sp,
                                    mybir.ActivationFunctionType.Exp,
                                    scale=0.125,
                                )
                                if r >= 0:
                                    # causal: zero the upper triangle of the
                                    # single diagonal 128x128 block (keep
                                    # where in-block i >= j)
                                    nc.gpsimd.affine_select(
                                        out=ex[:, 0:P],
                                        in_=ex[:, 0:P],
                                        compare_op=mybir.AluOpType.is_ge,
                                        fill=0.0,
                                        base=0,
                                        channel_multiplier=-1,
                                        pattern=[[1, P]],
                                    )
                                exs.append(ex)
                            for half in range(2):
                                h_ = 2 * hp + half
                                for ic in range(JPI):
                                    if ic < r:
                                        continue  # chunk entirely above diag
                                    c0 = ic * P - off
                                    # one psum accumulation group per bank:
                                    # start clears the whole 2KB zero region,
                                    # so only the first matmul starts and only
                                    # the last stops; first-touch writes of
                                    # other ic regions overwrite via the
                                    # per-element has_written bit.
                                    nc.tensor.matmul(
                                        ops_pair[half][:, ic, :],
                                        exs[half][:, c0 : c0 + P],
                                        v_aug[:, jt, h_, :],
                                        start=(jt == 0 and ic == 0),
                                        stop=(jt == njt - 1 and ic == JPI - 1),
                                    )
                        for half in range(2):
                            h_ = 2 * hp + half
                            rc = recip_pool.tile(
                                [P, JPI], F32, tag="recip", name="rc"
                            )
                            nc.vector.reciprocal(rc, ops_pair[half][:, :, HS])
                            nc.vector.tensor_tensor(
                                out_sb[:, :, h_ * HS : (h_ + 1) * HS],
                                ops_pair[half][:, :, 0:HS],
                                rc[:, :, None].to_broadcast((P, JPI, HS)),
                                mybir.AluOpType.mult,
                            )
                    for it in range(JPI):
                        git = I * JPI + it
                        nc.vector.tensor_add(
                            out=out_sb[:, it, :],
                            in0=out_sb[:, it, :],
                            in1=bias_v,
                        )
                        nc.sync.dma_start(
                            out_d[git * P : (git + 1) * P, :],
                            out_sb[:, it, :],
                        )

                if interleave:
                    for tb in range(NTB):
                        p1_tblock(tb)
                        p2_iblock(tb)
                else:
                    for tb in range(NTB):
                        p1_tblock(tb)
                    for I in range(NI):
                        p2_iblock(I)

    nc.compile()
    return nc


def get_nc():
    if "nc" not in _CACHE:
        _CACHE["nc"] = _build_nc()
    return _CACHE["nc"]


def shard_inputs(x, W_qkv, b_qkv):
    """Split full inputs into the 8 per-core input maps (host-side
    transpose + bf16 cast; the device does no transposes)."""
    import concourse.mybir as mybir

    bf16 = mybir.dt.np(mybir.dt.bfloat16)
    xT_by_b = {}
    for b_ in range(B):
        xT_by_b[b_] = np.ascontiguousarray(x[b_].T).astype(bf16)
    w_by_g = {}
    for g in range(2):
        rq = slice(g * F, (g + 1) * F)
        rk = slice(E + g * F, E + (g + 1) * F)
        rv = slice(2 * E + g * F, 2 * E + (g + 1) * F)
        w_c = np.concatenate([W_qkv[rq], W_qkv[rk], W_qkv[rv]], axis=0)
        b_c = np.concatenate([b_qkv[rq], b_qkv[rk], b_qkv[rv]], axis=0)
        w_by_g[g] = (
            np.ascontiguousarray(w_c.T).astype(bf16),
            np.ascontiguousarray(b_c, dtype=np.float32),
        )
    in_maps = []
    for c in range(N_CORES):
        b_, g = c // 2, c % 2
        wT, b_c = w_by_g[g]
        in_maps.append({"xT": xT_by_b[b_], "w": wT, "b": b_c})
    return in_maps


def gather_output(results):
    """Assemble per-core [T, F] outputs into the full [B, T, E] output."""
    out = np.empty((B, T, E), dtype=np.float32)
    for c in range(N_CORES):
        b_, g = c // 2, c % 2
        out[b_, :, g * F : (g + 1) * F] = results[c]["out"]
    return out


def kernel(x, W_qkv, b_qkv):
    from concourse.bass_utils import run_bass_kernel_spmd

    x = np.asarray(x, dtype=np.float32)
    W_qkv = np.asarray(W_qkv, dtype=np.float32)
    b_qkv = np.asarray(b_qkv, dtype=np.float32)
    in_maps = shard_inputs(x, W_qkv, b_qkv)
    res = run_bass_kernel_spmd(get_nc(), in_maps, core_ids=list(range(N_CORES)))
    return gather_output(res.results)
